# revision 1
# baseline (speedup 1.0000x reference)
"""Cross-attention Trainium2 kernel.

Problem: B=8, SQ=SKV=2048, HIDDEN=256, fp32.
  Q = query @ Wq.T + bq ; K = key @ Wk.T + bk ; V = value @ Wv.T + bv
  out = softmax(Q @ K.T / sqrt(128)) @ V

Sharding: data-parallel over batch — one batch element per NeuronCore,
8 cores, no collectives. Each core runs an identical program on its
batch slice. Activations are passed to the device in [d, s] layout
(transposed on the host as part of sharding/layout prep) because the PE
contracts the partition dim, so the d=256 projections need d on
partitions; weights are likewise passed pre-transposed [d, e].

Per-core pipeline:
  P:  projections.  K^T[e,k] and Q^T[e,q] come out of the PE directly
      in transposed layout (bias fused into the ACT PSUM->SBUF
      eviction).  V stays natural [k,e]; bv is added by DVE with a
      partition-broadcast bias tile into V' which carries two extra
      all-ones columns (col 256 = softmax denominator, col 257 pads the
      fp32r matmul free dim to an even size).
  S:  S^T[k,q] = (K^T).T @ Q^T accumulated over e, per 512-wide q
      block.  exp(x/SCALE) fused into the ACT eviction.  No
      max-subtraction: scores are ~N(0,0.5) by construction, exp is
      safe in fp32.
  A:  numerator AND denominator in one matmul: U.T @ V' with the ones
      column giving psum col 256 = sum_k exp.  Final: out =
      psum[:, :256] * reciprocal(col 256) on ACT (bv is inside V', so
      the division yields attention-with-bias exactly).

All matmuls run as float32r (full-rate 4-byte PE path, ~tf32 multiply
precision, fp32 PSUM accumulation); every SBUF operand consumed by an
fp32r matmul is produced by a rounding instruction as walrus requires.
"""

import numpy as np

B, SQ, SKV, H = 8, 2048, 2048, 256
SCALE = float(np.sqrt(H / 2.0))
N_CORES = 8

P = 128          # partitions
DC = H // P      # d chunks (2)
EC = H // P      # e chunks (2)
NB = SQ // 512   # 512-row seq blocks (4)
KC = SKV // P    # k chunks (16)

_CACHE: dict = {}


def _emit(ctx, tc, aps):
    from concourse import mybir

    nc = tc.nc
    f32 = mybir.dt.float32
    f32r = mybir.dt.float32r
    AF = mybir.ActivationFunctionType
    queryT, keyT, valueT, wqT, wkT, wvT, bq2, bk2, bvr, out = aps
    inv_scale = 1.0 / SCALE
    r = lambda ap: ap.bitcast(f32r)  # full-rate PE path for 4-byte data

    const_pool = ctx.enter_context(tc.tile_pool(name="const", bufs=1))
    kin_pool = ctx.enter_context(tc.tile_pool(name="kin", bufs=3))
    qin_pool = ctx.enter_context(tc.tile_pool(name="qin", bufs=3))
    ktv_pool = ctx.enter_context(tc.tile_pool(name="ktv", bufs=1))
    qt_pool = ctx.enter_context(tc.tile_pool(name="qt", bufs=2))
    u_pool = ctx.enter_context(tc.tile_pool(name="u", bufs=9))
    out_pool = ctx.enter_context(tc.tile_pool(name="outp", bufs=3))
    rec_pool = ctx.enter_context(tc.tile_pool(name="rec", bufs=3))
    ps_a = ctx.enter_context(tc.tile_pool(name="ps_a", bufs=2, space="PSUM"))
    ps_v = ctx.enter_context(tc.tile_pool(name="ps_v", bufs=2, space="PSUM"))
    ps_av = ctx.enter_context(tc.tile_pool(name="ps_av", bufs=2, space="PSUM"))

    # ---- constants ----
    # weights as [d_part, dc, e]; the DRAM tensors are declared float32r so
    # the DMA'd tiles can feed fp32r matmuls directly.
    def load_weight(name, src_ap):
        w = const_pool.tile([P, DC, H], f32r, tag=name)
        nc.scalar.dma_start(w, src_ap.rearrange("(c p) e -> p c e", p=P))
        return w

    wk_sb = load_weight("wk", wkT)
    wv_sb = load_weight("wv", wvT)
    wq_sb = load_weight("wq", wqT)

    bq_sb = const_pool.tile([P, EC], f32)
    nc.scalar.dma_start(bq_sb, bq2.rearrange("c p -> p c"))
    bk_sb = const_pool.tile([P, EC], f32)
    nc.scalar.dma_start(bk_sb, bk2.rearrange("c p -> p c"))
    bv_row = const_pool.tile([1, H], f32)
    nc.scalar.dma_start(bv_row, bvr)
    bv_rep = const_pool.tile([P, H], f32)
    nc.gpsimd.partition_broadcast(bv_rep, bv_row)

    # ---- persistent per-core tensors ----
    KT = ktv_pool.tile([P, EC, SKV], f32)      # [e_part, ec, k]
    # V' carries 2 extra columns of ones: col 256 is the softmax
    # denominator; col 257 only pads the fp32r matmul free dim to an even
    # size (odd N fails walrus codegen).  memset can't produce fp32r, so
    # write the ones via tensor_scalar (in*0 + 1).
    Vp = ktv_pool.tile([P, KC, H + 2], f32)    # [k_part, kc, e | ones ones]
    for kc in range(KC):
        nc.vector.tensor_scalar(
            r(Vp[:, kc, H:H + 2]), bv_rep[:, 0:2], 0.0, 1.0,
            mybir.AluOpType.mult, mybir.AluOpType.add,
        )

    def load_T(src, blk, dma, tag, pool):
        """DMA a 512-col block of a [H, seq] dram tensor into a
        [d_part, dc, 512] SBUF tile (2KB-contiguous rows per partition)."""
        t = pool.tile([P, DC, 512], f32r, tag=tag)
        dma.dma_start(
            t, src[:, blk * 512:(blk + 1) * 512].rearrange("(c p) s -> p c s", p=P)
        )
        return t

    # ---- key: project into KT ----
    for blk in range(NB):
        ktr = load_T(keyT, blk, nc.sync if blk % 2 == 0 else nc.scalar, "kin", kin_pool)
        pk = ps_a.tile([P, 1024], f32, tag="ps_a")
        for ec in range(EC):
            for dc in range(DC):
                nc.tensor.matmul(
                    pk[:, ec * 512:(ec + 1) * 512],
                    lhsT=r(wk_sb[:, dc, ec * P:(ec + 1) * P]),
                    rhs=r(ktr[:, dc, :]),
                    start=(dc == 0),
                    stop=(dc == DC - 1),
                )
        for ec in range(EC):
            nc.vector.tensor_scalar(
                r(KT[:, ec, blk * 512:(blk + 1) * 512]),
                pk[:, ec * 512:(ec + 1) * 512],
                bk_sb[:, ec:ec + 1], None, mybir.AluOpType.add,
            )

    # ---- value: project into Vp (+bv) ----
    for blk in range(NB):
        vtr = load_T(valueT, blk, nc.scalar, "vin", kin_pool)
        for j in range(4):
            kc = blk * 4 + j
            pv = ps_v.tile([P, H], f32, tag="ps_v")
            for dc in range(DC):
                nc.tensor.matmul(
                    pv,
                    lhsT=r(vtr[:, dc, j * P:(j + 1) * P]),
                    rhs=r(wv_sb[:, dc, :]),
                    start=(dc == 0),
                    stop=(dc == DC - 1),
                )
            nc.vector.tensor_add(r(Vp[:, kc, 0:H]), pv, bv_rep)

    # ---- query blocks: project, scores+exp, AV, finalize ----
    for qb in range(NB):
        qtr = load_T(queryT, qb, nc.sync, "qin", qin_pool)
        pq = ps_a.tile([P, 1024], f32, tag="ps_a")
        for ec in range(EC):
            for dc in range(DC):
                nc.tensor.matmul(
                    pq[:, ec * 512:(ec + 1) * 512],
                    lhsT=r(wq_sb[:, dc, ec * P:(ec + 1) * P]),
                    rhs=r(qtr[:, dc, :]),
                    start=(dc == 0),
                    stop=(dc == DC - 1),
                )
        qt = qt_pool.tile([P, 1024], f32, tag="qt")   # [e_part, ec*512 + q]
        for ec in range(EC):
            nc.vector.tensor_scalar(
                r(qt[:, ec * 512:(ec + 1) * 512]),
                pq[:, ec * 512:(ec + 1) * 512],
                bq_sb[:, ec:ec + 1], None, mybir.AluOpType.add,
            )

        # scores S^T[k, q] for this q block, exp'ed into U tiles
        us = []
        for kp in range(KC // 2):
            pst = ps_a.tile([P, 1024], f32, tag="ps_a")
            for hh in range(2):
                kc = kp * 2 + hh
                for ec in range(EC):
                    nc.tensor.matmul(
                        pst[:, hh * 512:(hh + 1) * 512],
                        lhsT=r(KT[:, ec, kc * P:(kc + 1) * P]),
                        rhs=r(qt[:, ec * 512:(ec + 1) * 512]),
                        start=(ec == 0),
                        stop=(ec == EC - 1),
                    )
            u2 = u_pool.tile([P, 1024], f32, tag="u2")
            nc.scalar.activation(r(u2), pst, AF.Exp, scale=inv_scale)
            us.append(u2)

        # attention output: numerator + denominator in one accumulation
        for qs in range(4):
            pav = ps_av.tile([P, H + 2], f32, tag="ps_av")
            for kc in range(KC):
                u2 = us[kc // 2]
                off = (kc % 2) * 512
                nc.tensor.matmul(
                    pav,
                    lhsT=r(u2[:, off + qs * P: off + (qs + 1) * P]),
                    rhs=r(Vp[:, kc, :]),
                    start=(kc == 0),
                    stop=(kc == KC - 1),
                )
            ot = out_pool.tile([P, H], f32, tag="ot")
            # NB: vector.tensor_scalar_mul reading a scalar that DVE's
            # reciprocal just produced crashes the device (observed
            # NRT_EXEC_UNIT_UNRECOVERABLE); route the multiply through
            # ACT instead so the scalar consumer sits on another engine.
            rec = rec_pool.tile([P, 1], f32, tag="rec")
            nc.vector.reciprocal(rec, pav[:, H:H + 1])
            nc.scalar.activation(ot, pav[:, 0:H], AF.Copy, scale=rec)
            nc.sync.dma_start(
                out[qb * 512 + qs * P: qb * 512 + (qs + 1) * P, :], ot
            )


def _build():
    from contextlib import ExitStack

    import concourse.tile as tile
    from concourse import bacc, mybir

    f32 = mybir.dt.float32
    nc = bacc.Bacc(
        "TRN2", target_bir_lowering=False, debug=False, num_devices=N_CORES
    )
    queryT = nc.dram_tensor("queryT", [H, SQ], mybir.dt.float32r, kind="ExternalInput").ap()
    keyT = nc.dram_tensor("keyT", [H, SKV], mybir.dt.float32r, kind="ExternalInput").ap()
    valueT = nc.dram_tensor("valueT", [H, SKV], mybir.dt.float32r, kind="ExternalInput").ap()
    wqT = nc.dram_tensor("wqT", [H, H], mybir.dt.float32r, kind="ExternalInput").ap()
    wkT = nc.dram_tensor("wkT", [H, H], mybir.dt.float32r, kind="ExternalInput").ap()
    wvT = nc.dram_tensor("wvT", [H, H], mybir.dt.float32r, kind="ExternalInput").ap()
    bq2 = nc.dram_tensor("bq2", [EC, P], f32, kind="ExternalInput").ap()
    bk2 = nc.dram_tensor("bk2", [EC, P], f32, kind="ExternalInput").ap()
    bvr = nc.dram_tensor("bvr", [1, H], f32, kind="ExternalInput").ap()
    out = nc.dram_tensor("out", [SQ, H], f32, kind="ExternalOutput").ap()

    aps = (queryT, keyT, valueT, wqT, wkT, wvT, bq2, bk2, bvr, out)
    with tile.TileContext(nc) as tc, ExitStack() as ctx:
        _emit(ctx, tc, aps)
    nc.compile()
    return nc


def _get_nc():
    if "nc" not in _CACHE:
        _CACHE["nc"] = _build()
    return _CACHE["nc"]


def _in_maps(query, key, value, Wq, bq, Wk, bk, Wv, bv):
    q = np.asarray(query, np.float32)
    k = np.asarray(key, np.float32)
    v = np.asarray(value, np.float32)
    # [B, s, d] -> [B, d, s] layout prep for the device (contraction dim on
    # partitions), done host-side as part of sharding.
    qT = np.ascontiguousarray(q.transpose(0, 2, 1))
    kT = np.ascontiguousarray(k.transpose(0, 2, 1))
    vT = np.ascontiguousarray(v.transpose(0, 2, 1))
    wqT = np.ascontiguousarray(np.asarray(Wq, np.float32).T)
    wkT = np.ascontiguousarray(np.asarray(Wk, np.float32).T)
    wvT = np.ascontiguousarray(np.asarray(Wv, np.float32).T)
    bq2 = np.ascontiguousarray(np.asarray(bq, np.float32).reshape(EC, P))
    bk2 = np.ascontiguousarray(np.asarray(bk, np.float32).reshape(EC, P))
    bvr = np.ascontiguousarray(np.asarray(bv, np.float32).reshape(1, H))
    maps = []
    for b in range(B):
        maps.append(
            {
                "queryT": qT[b],
                "keyT": kT[b],
                "valueT": vT[b],
                "wqT": wqT,
                "wkT": wkT,
                "wvT": wvT,
                "bq2": bq2,
                "bk2": bk2,
                "bvr": bvr,
            }
        )
    return maps


def _run(in_maps, trace=False, **kw):
    import concourse.bass_utils as bass_utils

    if trace:
        # zero-egress container: skip the artifact upload step
        bass_utils.upload_artifacts = lambda tmpdir: f"local://{tmpdir}"
    nc = _get_nc()
    return bass_utils.run_bass_kernel_spmd(
        nc, in_maps, list(range(N_CORES)), trace=trace, **kw
    )


def kernel(query, key, value, Wq, bq, Wk, bk, Wv, bv):
    res = _run(_in_maps(query, key, value, Wq, bq, Wk, bk, Wv, bv))
    return np.stack([res.results[b]["out"] for b in range(B)], axis=0)



# revision 5
# speedup vs baseline: 1.1902x; 1.1902x over previous
"""Cross-attention Trainium2 kernel (v2: bf16 + fp8-DoubleRow scores).

Problem: B=8, SQ=SKV=2048, HIDDEN=256, fp32.
  Q = query @ Wq.T + bq ; K = key @ Wk.T + bk ; V = value @ Wv.T + bv
  out = softmax(Q @ K.T / sqrt(128)) @ V

Sharding: data-parallel over batch - one batch element per NeuronCore,
8 cores, no collectives.

Numerics (validated against the reference on CPU, harness gate 2e-2):
  inputs/weights bf16, projections bf16, Q/K quantized to fp8e4m3 for
  the scores matmul (fp8 DoubleRow, 256-deep contraction per pass),
  exp computed partly on ACT (exact LUT) and partly on DVE via the
  Schraudolph int16 bit trick (bits of bf16(exp(x)) ~ x*2^7/ln2 +
  16248.6), U and V kept in bf16 for the AV matmul.  Simulated rel
  err ~1.3e-2.

Per-core pipeline:
  P:  K/V/Q projections in bf16.  K^T evicted to fp8 [e,k] layout
      (bias fused in the DVE eviction), V kept natural [k,e] in bf16
      with 2 extra ones columns (col 256 = softmax denominator,
      col 257 pads free dim even), Q^T evicted to fp8 [e,q] per block.
  S:  S^T[k,q] per 512-q block via fp8 DoubleRow: one matmul per
      128-k chunk contracts all 256 e in a single pass.
  E:  exp: ACT tiles 0..4 of each block (Exp LUT, scale=1/SCALE,
      bf16 out), DVE tiles 5..7 (tensor_scalar bit trick, int16 out
      bitcast to bf16).
  A:  numerator+denominator in one bf16 accumulation over 16 k-chunks
      into [q,258] PSUM; finalize = DVE reciprocal of col 256 + ACT
      copy-scale (DVE reading its own just-produced reciprocal crashes
      the device, so the multiply stays on ACT).
"""

import numpy as np

B, SQ, SKV, H = 8, 2048, 2048, 256
SCALE = float(np.sqrt(H / 2.0))
N_CORES = 8

P = 128          # partitions
DC = H // P      # d chunks (2)
EC = H // P      # e chunks (2)
NB = SQ // 512   # 512-row seq blocks (4)
KC = SKV // P    # k chunks (16)
NPAIR = KC // 2  # kc pairs per q block (8)
N_SCHR = 3       # pairs per block on DVE via Schraudolph (of 8)

# Schraudolph constants for bf16-bit output: bits = x*(2^7/ln2) + B16
# B16 = 127*2^7 - 0.0579*2^7 (zero-mean tuning) + 0.5 (int16 convert
# truncates toward zero; all values positive)
SCHR_S = 128.0 / float(np.log(2.0))
SCHR_B = 127.0 * 128.0 - 7.40 + 0.5

_CACHE: dict = {}


def _emit(ctx, tc, aps):
    from concourse import mybir

    nc = tc.nc
    f32 = mybir.dt.float32
    bf16 = mybir.dt.bfloat16
    fp8 = mybir.dt.float8e4
    i16 = mybir.dt.int16
    AF = mybir.ActivationFunctionType
    DR = mybir.MatmulPerfMode.DoubleRow
    queryT, keyT, valueT, wqT, wkT, wvT, bq2, bk2, bvr, out = aps
    inv_scale = 1.0 / SCALE

    const_pool = ctx.enter_context(tc.tile_pool(name="const", bufs=1))
    kin_pool = ctx.enter_context(tc.tile_pool(name="kin", bufs=3))
    qin_pool = ctx.enter_context(tc.tile_pool(name="qin", bufs=2))
    ktv_pool = ctx.enter_context(tc.tile_pool(name="ktv", bufs=1))
    qt_pool = ctx.enter_context(tc.tile_pool(name="qt", bufs=2))
    u_pool = ctx.enter_context(tc.tile_pool(name="u", bufs=18))
    out_pool = ctx.enter_context(tc.tile_pool(name="outp", bufs=4))
    rec_pool = ctx.enter_context(tc.tile_pool(name="rec", bufs=4))
    # PSUM budget (8 banks of 2KB): pst 2x2 + proj 2x1 + pav 2x1 banks
    ps_s = ctx.enter_context(tc.tile_pool(name="ps_s", bufs=2, space="PSUM"))
    ps_p = ctx.enter_context(tc.tile_pool(name="ps_p", bufs=2, space="PSUM"))
    ps_av = ctx.enter_context(tc.tile_pool(name="ps_av", bufs=2, space="PSUM"))

    # ---- constants ----
    def load_weight(name, src_ap):
        w = const_pool.tile([P, DC, H], bf16, tag=name)
        nc.sync.dma_start(w, src_ap.rearrange("(c p) e -> p c e", p=P))
        return w

    wk_sb = load_weight("wk", wkT)
    wv_sb = load_weight("wv", wvT)
    wq_sb = load_weight("wq", wqT)

    bq_sb = const_pool.tile([P, EC], f32)
    nc.sync.dma_start(bq_sb, bq2.rearrange("c p -> p c"))
    bk_sb = const_pool.tile([P, EC], f32)
    nc.sync.dma_start(bk_sb, bk2.rearrange("c p -> p c"))
    bv_row = const_pool.tile([1, H], f32)
    nc.sync.dma_start(bv_row, bvr)
    bv_rep = const_pool.tile([P, H], f32)
    nc.gpsimd.partition_broadcast(bv_rep, bv_row)

    # ---- persistent per-core tensors ----
    KT8 = ktv_pool.tile([P, EC, SKV], fp8)      # [e_part, ec, k]
    Vb = ktv_pool.tile([P, KC, H + 2], bf16)    # [k_part, kc, e | one one]
    for kc in range(KC):
        nc.vector.tensor_scalar(
            Vb[:, kc, H:H + 2], bv_rep[:, 0:2], 0.0, 1.0,
            mybir.AluOpType.mult, mybir.AluOpType.add,
        )

    def load_T(src, blk, dma, tag, pool):
        t = pool.tile([P, DC, 512], bf16, tag=tag)
        dma.dma_start(
            t, src[:, blk * 512:(blk + 1) * 512].rearrange("(c p) s -> p c s", p=P)
        )
        return t

    # ---- key: project into KT8 (fp8, bias fused) ----
    for blk in range(NB):
        ktr = load_T(keyT, blk, nc.gpsimd, "kin", kin_pool)
        for ec in range(EC):
            pk = ps_p.tile([P, 512], f32, tag="ps_p")
            for dc in range(DC):
                nc.tensor.matmul(
                    pk,
                    lhsT=wk_sb[:, dc, ec * P:(ec + 1) * P],
                    rhs=ktr[:, dc, :],
                    start=(dc == 0),
                    stop=(dc == DC - 1),
                )
            nc.vector.tensor_scalar(
                KT8[:, ec, blk * 512:(blk + 1) * 512],
                pk, bk_sb[:, ec:ec + 1], None, mybir.AluOpType.add,
            )

    # ---- value: project into Vb (+bv, bf16) ----
    for blk in range(NB):
        vtr = load_T(valueT, blk, nc.gpsimd, "vin", kin_pool)
        for j in range(4):
            kc = blk * 4 + j
            pvt = ps_p.tile([P, 512], f32, tag="ps_p")
            pv = pvt[:, 0:H]
            for dc in range(DC):
                nc.tensor.matmul(
                    pv,
                    lhsT=vtr[:, dc, j * P:(j + 1) * P],
                    rhs=wv_sb[:, dc, :],
                    start=(dc == 0),
                    stop=(dc == DC - 1),
                )
            nc.vector.tensor_add(Vb[:, kc, 0:H], pv, bv_rep)

    # ---- query blocks: project, scores+exp, AV, finalize ----
    # Software pipeline: emit proj+scores+exp for qb, then AV+finalize
    # for qb-1, so the PE runs AV(qb-1) while ACT/DVE exp(qb).
    pending = None  # (qb, us) awaiting AV+finalize

    def emit_av(qb, us):
        for qs in range(4):
            pav = ps_av.tile([P, H + 2], f32, tag="pav")
            for g in range(NPAIR):
                u = us[g]
                for hh in range(2):
                    kc = 2 * g + hh
                    nc.tensor.matmul(
                        pav,
                        lhsT=u[:, hh * 512 + qs * P: hh * 512 + (qs + 1) * P],
                        rhs=Vb[:, kc, :],
                        start=(kc == 0),
                        stop=(kc == KC - 1),
                    )
            rec = rec_pool.tile([P, 1], f32, tag="rec")
            nc.vector.reciprocal(rec, pav[:, H:H + 1])
            ot = out_pool.tile([P, H], f32, tag="ot")
            nc.scalar.activation(ot, pav[:, 0:H], AF.Copy, scale=rec)
            nc.sync.dma_start(
                out[qb * 512 + qs * P: qb * 512 + (qs + 1) * P, :], ot
            )

    for qb in range(NB):
        qtr = load_T(queryT, qb, nc.sync, "qin", qin_pool)
        qt8 = qt_pool.tile([P, EC, 512], fp8, tag="qt8")
        for ec in range(EC):
            pq = ps_p.tile([P, 512], f32, tag="ps_p")
            for dc in range(DC):
                nc.tensor.matmul(
                    pq,
                    lhsT=wq_sb[:, dc, ec * P:(ec + 1) * P],
                    rhs=qtr[:, dc, :],
                    start=(dc == 0),
                    stop=(dc == DC - 1),
                )
            nc.vector.tensor_scalar(
                qt8[:, ec, :], pq, bq_sb[:, ec:ec + 1], None, mybir.AluOpType.add,
            )

        # scores S^T[k,q]: one DoubleRow matmul per 128-k chunk
        us = []
        for g in range(NPAIR):
            pst = ps_s.tile([P, 1024], f32, tag="pst")
            for hh in range(2):
                kc = 2 * g + hh
                nc.tensor.matmul(
                    pst[:, hh * 512:(hh + 1) * 512],
                    lhsT=KT8[:, :, kc * P:(kc + 1) * P],
                    rhs=qt8,
                    start=True,
                    stop=True,
                    perf_mode=DR,
                )
            if g < NPAIR - N_SCHR:
                u = u_pool.tile([P, 1024], bf16, tag="u")
                nc.scalar.activation(u, pst, AF.Exp, scale=inv_scale)
            else:
                u16 = u_pool.tile([P, 1024], i16, tag="u16")
                nc.vector.tensor_scalar(
                    u16, pst, SCHR_S * inv_scale, SCHR_B,
                    mybir.AluOpType.mult, mybir.AluOpType.add,
                )
                u = u16.bitcast(bf16)
            us.append(u)

        if pending is not None:
            emit_av(*pending)
        pending = (qb, us)

    emit_av(*pending)


def _build():
    from contextlib import ExitStack

    import concourse.tile as tile
    from concourse import bacc, mybir

    f32 = mybir.dt.float32
    bf16 = mybir.dt.bfloat16
    nc = bacc.Bacc(
        "TRN2", target_bir_lowering=False, debug=False, num_devices=N_CORES
    )
    queryT = nc.dram_tensor("queryT", [H, SQ], bf16, kind="ExternalInput").ap()
    keyT = nc.dram_tensor("keyT", [H, SKV], bf16, kind="ExternalInput").ap()
    valueT = nc.dram_tensor("valueT", [H, SKV], bf16, kind="ExternalInput").ap()
    wqT = nc.dram_tensor("wqT", [H, H], bf16, kind="ExternalInput").ap()
    wkT = nc.dram_tensor("wkT", [H, H], bf16, kind="ExternalInput").ap()
    wvT = nc.dram_tensor("wvT", [H, H], bf16, kind="ExternalInput").ap()
    bq2 = nc.dram_tensor("bq2", [EC, P], f32, kind="ExternalInput").ap()
    bk2 = nc.dram_tensor("bk2", [EC, P], f32, kind="ExternalInput").ap()
    bvr = nc.dram_tensor("bvr", [1, H], f32, kind="ExternalInput").ap()
    out = nc.dram_tensor("out", [SQ, H], f32, kind="ExternalOutput").ap()

    aps = (queryT, keyT, valueT, wqT, wkT, wvT, bq2, bk2, bvr, out)
    with tile.TileContext(nc) as tc, ExitStack() as ctx:
        _emit(ctx, tc, aps)
    nc.compile()
    return nc


def _get_nc():
    if "nc" not in _CACHE:
        _CACHE["nc"] = _build()
    return _CACHE["nc"]


def _in_maps(query, key, value, Wq, bq, Wk, bk, Wv, bv):
    import ml_dtypes

    bf = ml_dtypes.bfloat16
    q = np.asarray(query, np.float32)
    k = np.asarray(key, np.float32)
    v = np.asarray(value, np.float32)
    # [B, s, d] -> [B, d, s] bf16 layout prep, done host-side as part of
    # sharding (contraction dim on partitions; bf16 halves the DMA bytes).
    qT = np.ascontiguousarray(q.transpose(0, 2, 1)).astype(bf)
    kT = np.ascontiguousarray(k.transpose(0, 2, 1)).astype(bf)
    vT = np.ascontiguousarray(v.transpose(0, 2, 1)).astype(bf)
    wqT = np.ascontiguousarray(np.asarray(Wq, np.float32).T).astype(bf)
    wkT = np.ascontiguousarray(np.asarray(Wk, np.float32).T).astype(bf)
    wvT = np.ascontiguousarray(np.asarray(Wv, np.float32).T).astype(bf)
    bq2 = np.ascontiguousarray(np.asarray(bq, np.float32).reshape(EC, P))
    bk2 = np.ascontiguousarray(np.asarray(bk, np.float32).reshape(EC, P))
    bvr = np.ascontiguousarray(np.asarray(bv, np.float32).reshape(1, H))
    maps = []
    for b in range(B):
        maps.append(
            {
                "queryT": qT[b],
                "keyT": kT[b],
                "valueT": vT[b],
                "wqT": wqT,
                "wkT": wkT,
                "wvT": wvT,
                "bq2": bq2,
                "bk2": bk2,
                "bvr": bvr,
            }
        )
    return maps


def _run(in_maps, trace=False, **kw):
    import concourse.bass_utils as bass_utils

    if trace:
        # zero-egress container: skip the artifact upload step
        bass_utils.upload_artifacts = lambda tmpdir: f"local://{tmpdir}"
    nc = _get_nc()
    return bass_utils.run_bass_kernel_spmd(
        nc, in_maps, list(range(N_CORES)), trace=trace, **kw
    )


def kernel(query, key, value, Wq, bq, Wk, bk, Wv, bv):
    res = _run(_in_maps(query, key, value, Wq, bq, Wk, bk, Wv, bv))
    return np.stack([res.results[b]["out"] for b in range(B)], axis=0)


# revision 12
# speedup vs baseline: 1.2415x; 1.0431x over previous
"""Cross-attention Trainium2 kernel (v3: bf16 + fp8-DoubleRow scores).

Problem: B=8, SQ=SKV=2048, HIDDEN=256, fp32.
  Q = query @ Wq.T + bq ; K = key @ Wk.T + bk ; V = value @ Wv.T + bv
  out = softmax(Q @ K.T / sqrt(128)) @ V

Sharding: data-parallel over batch - one batch element per NeuronCore,
8 cores, no collectives.

Numerics (validated vs the reference on CPU + HW, harness gate 2e-2):
  inputs/weights bf16, projections bf16, Q/K quantized to fp8e4m3 for
  the scores matmul (fp8 DoubleRow, 256-deep contraction per pass),
  exp partly on ACT (Exp LUT) and partly on DVE (Schraudolph int16
  bit trick: bits of bf16(exp(x)) ~ x*2^7/ln2 + 16248.6+0.5), U and V
  in bf16 for the AV matmul.  Measured HW rel err ~1.2e-2.

Layout/scheduling notes:
  - whole K/V/Q tensors DMA'd up front, one instruction each, spread
    over the SP and Pool DMA queues (per-block loads cost ~1us of
    descriptor generation each and serialized the first 15us).
  - PE emission order per q-block: projection(qb), AV(qb-1),
    scores(qb).  AV covers the DVE latency of the qt8 eviction so the
    PE never stalls at the block boundary (HAM re-throttle).
  - V projection lands 4 k-chunks in one [128,1024] PSUM tile and is
    evicted by a single DVE tensor_tensor (+bv, wide bias tile) per
    group: 4 instructions instead of 16.
  - softmax denominator = ones-columns of Vb (col 256; col 257 pads
    the free dim even); finalize = DVE reciprocal + ACT copy-scale
    (DVE reading its own just-produced reciprocal crashes the device).
"""

import numpy as np

B, SQ, SKV, H = 8, 2048, 2048, 256
SCALE = float(np.sqrt(H / 2.0))
N_CORES = 8

P = 128          # partitions
DC = H // P      # d chunks (2)
EC = H // P      # e chunks (2)
NB = SQ // 512   # 512-row seq blocks (4)
KC = SKV // P    # k chunks (16)
NPAIR = KC // 2  # kc pairs per q block (8)
N_SCHR = 3       # pairs per block on DVE via Schraudolph (of 8)

# Schraudolph constants for bf16-bit output (int16 convert truncates;
# +0.5 recenters; -7.40 zero-means the piecewise-linear error)
SCHR_S = 128.0 / float(np.log(2.0))
SCHR_B = 127.0 * 128.0 - 7.40 + 0.5

_CACHE: dict = {}


def _emit(ctx, tc, aps):
    from concourse import mybir

    nc = tc.nc
    f32 = mybir.dt.float32
    bf16 = mybir.dt.bfloat16
    fp8 = mybir.dt.float8e4
    i16 = mybir.dt.int16
    AF = mybir.ActivationFunctionType
    DR = mybir.MatmulPerfMode.DoubleRow
    queryT, keyT, valueT, wqT, wkT, wvT, bq2, bk2, bv4 = aps[:9]
    out = aps[9]
    inv_scale = 1.0 / SCALE

    const_pool = ctx.enter_context(tc.tile_pool(name="const", bufs=1))
    ktv_pool = ctx.enter_context(tc.tile_pool(name="ktv", bufs=1))
    qt_pool = ctx.enter_context(tc.tile_pool(name="qt", bufs=2))
    u_pool = ctx.enter_context(tc.tile_pool(name="u", bufs=18))
    out_pool = ctx.enter_context(tc.tile_pool(name="outp", bufs=4))
    rec_pool = ctx.enter_context(tc.tile_pool(name="rec", bufs=4))
    # PSUM budget (8 banks of 2KB): pst 2x2 + proj 2x1 + pav 2x1 banks
    ps_s = ctx.enter_context(tc.tile_pool(name="ps_s", bufs=2, space="PSUM"))
    ps_p = ctx.enter_context(tc.tile_pool(name="ps_p", bufs=2, space="PSUM"))
    ps_av = ctx.enter_context(tc.tile_pool(name="ps_av", bufs=2, space="PSUM"))

    # ---- bulk input DMAs, all issued at t=0 ----
    # One DMA per tensor, all on the SP queue so they run SEQUENTIALLY:
    # each gets the full HBM bandwidth, and kin (the first consumer's
    # input) lands first instead of sharing bandwidth three ways.
    kin = const_pool.tile([P, DC, SKV], bf16, tag="kin")
    nc.sync.dma_start(kin, keyT.rearrange("(c p) s -> p c s", p=P))
    vin = const_pool.tile([P, DC, SKV], bf16, tag="vin")
    nc.sync.dma_start(vin, valueT.rearrange("(c p) s -> p c s", p=P))
    qin = const_pool.tile([P, DC, SQ], bf16, tag="qin")
    nc.sync.dma_start(qin, queryT.rearrange("(c p) s -> p c s", p=P))

    def load_weight(name, src_ap):
        w = const_pool.tile([P, DC, H], bf16, tag=name)
        nc.scalar.dma_start(w, src_ap.rearrange("(c p) e -> p c e", p=P))
        return w

    wk_sb = load_weight("wk", wkT)
    wv_sb = load_weight("wv", wvT)
    wq_sb = load_weight("wq", wqT)

    bq_sb = const_pool.tile([P, EC], f32)
    nc.scalar.dma_start(bq_sb, bq2.rearrange("c p -> p c"))
    bk_sb = const_pool.tile([P, EC], f32)
    nc.scalar.dma_start(bk_sb, bk2.rearrange("c p -> p c"))
    bv_row = const_pool.tile([1, 4 * H], f32)
    nc.scalar.dma_start(bv_row, bv4)
    bv_rep = const_pool.tile([P, 4 * H], f32)
    nc.gpsimd.partition_broadcast(bv_rep, bv_row)

    # ---- persistent per-core tensors ----
    KT8 = ktv_pool.tile([P, EC, SKV], fp8)      # [e_part, ec, k]
    Vb = ktv_pool.tile([P, KC, H + 2], bf16)    # [k_part, kc, e | one one]
    for kc in range(KC):
        nc.vector.tensor_scalar(
            Vb[:, kc, H:H + 2], bv_rep[:, 0:2], 0.0, 1.0,
            mybir.AluOpType.mult, mybir.AluOpType.add,
        )

    # ---- query blocks: pipelined proj -> AV(prev) -> scores -> exp ----
    def emit_proj(qb):
        qt8 = qt_pool.tile([P, EC, 512], fp8, tag="qt8")
        for ec in range(EC):
            pq = ps_p.tile([P, 512], f32, tag="ps_p")
            for dc in range(DC):
                nc.tensor.matmul(
                    pq,
                    lhsT=wq_sb[:, dc, ec * P:(ec + 1) * P],
                    rhs=qin[:, dc, qb * 512:(qb + 1) * 512],
                    start=(dc == 0),
                    stop=(dc == DC - 1),
                )
            nc.vector.tensor_scalar(
                qt8[:, ec, :], pq, bq_sb[:, ec:ec + 1], None, mybir.AluOpType.add,
            )
        return qt8

    def emit_scores_exp(qb, qt8):
        us = []
        for g in range(NPAIR):
            pst = ps_s.tile([P, 1024], f32, tag="pst")
            for hh in range(2):
                kc = 2 * g + hh
                nc.tensor.matmul(
                    pst[:, hh * 512:(hh + 1) * 512],
                    lhsT=KT8[:, :, kc * P:(kc + 1) * P],
                    rhs=qt8,
                    start=True,
                    stop=True,
                    perf_mode=DR,
                )
            if g < NPAIR - N_SCHR:
                u = u_pool.tile([P, 1024], bf16, tag="u")
                nc.scalar.activation(u, pst, AF.Exp, scale=inv_scale)
            else:
                u16 = u_pool.tile([P, 1024], i16, tag="u16")
                nc.vector.tensor_scalar(
                    u16, pst, SCHR_S * inv_scale, SCHR_B,
                    mybir.AluOpType.mult, mybir.AluOpType.add,
                )
                u = u16.bitcast(bf16)
            us.append(u)
        return us

    def emit_av(qb, us):
        for qs in range(4):
            pav = ps_av.tile([P, H + 2], f32, tag="pav")
            for g in range(NPAIR):
                u = us[g]
                for hh in range(2):
                    kc = 2 * g + hh
                    nc.tensor.matmul(
                        pav,
                        lhsT=u[:, hh * 512 + qs * P: hh * 512 + (qs + 1) * P],
                        rhs=Vb[:, kc, :],
                        start=(kc == 0),
                        stop=(kc == KC - 1),
                    )
            rec = rec_pool.tile([P, 1], f32, tag="rec")
            nc.vector.reciprocal(rec, pav[:, H:H + 1])
            ot = out_pool.tile([P, H], f32, tag="ot")
            nc.scalar.activation(ot, pav[:, 0:H], AF.Copy, scale=rec)
            nc.sync.dma_start(
                out[qb * 512 + qs * P: qb * 512 + (qs + 1) * P, :], ot
            )

    # ---- key: project into KT8 (fp8, bias fused) ----
    for blk in range(NB):
        cols = slice(blk * 512, (blk + 1) * 512)
        for ec in range(EC):
            pk = ps_p.tile([P, 512], f32, tag="ps_p")
            for dc in range(DC):
                nc.tensor.matmul(
                    pk,
                    lhsT=wk_sb[:, dc, ec * P:(ec + 1) * P],
                    rhs=kin[:, dc, cols],
                    start=(dc == 0),
                    stop=(dc == DC - 1),
                )
            nc.vector.tensor_scalar(
                KT8[:, ec, cols],
                pk, bk_sb[:, ec:ec + 1], None, mybir.AluOpType.add,
            )

    # ---- Q projection of block 0, early: its DVE eviction drains
    # while the PE runs the V projection, so scores(0) start stall-free
    qt8_next = emit_proj(0)

    # ---- value: project into Vb (+bv), 4 k-chunks per PSUM tile ----
    for blk in range(NB):
        pv4 = ps_s.tile([P, 1024], f32, tag="pst")
        for j in range(4):
            kc = blk * 4 + j
            for dc in range(DC):
                nc.tensor.matmul(
                    pv4[:, j * H:(j + 1) * H],
                    lhsT=vin[:, dc, kc * P:(kc + 1) * P],
                    rhs=wv_sb[:, dc, :],
                    start=(dc == 0),
                    stop=(dc == DC - 1),
                )
        nc.vector.tensor_add(
            Vb[:, blk * 4:(blk + 1) * 4, 0:H], pv4, bv_rep,
        )

    # Pipeline: qt8(qb) is always evicted one iteration ahead, so
    # scores(qb) start with no DVE wait; AV(qb-1) fills the PE while
    # ACT/DVE chew through exp(qb).
    pending = None  # (qb, us) awaiting AV+finalize
    for qb in range(NB):
        qt8 = qt8_next
        us = emit_scores_exp(qb, qt8)
        if qb + 1 < NB:
            qt8_next = emit_proj(qb + 1)
        if pending is not None:
            emit_av(*pending)
        pending = (qb, us)
    emit_av(*pending)


def _build():
    from contextlib import ExitStack

    import concourse.tile as tile
    from concourse import bacc, mybir

    f32 = mybir.dt.float32
    bf16 = mybir.dt.bfloat16
    nc = bacc.Bacc(
        "TRN2", target_bir_lowering=False, debug=False, num_devices=N_CORES
    )
    queryT = nc.dram_tensor("queryT", [H, SQ], bf16, kind="ExternalInput").ap()
    keyT = nc.dram_tensor("keyT", [H, SKV], bf16, kind="ExternalInput").ap()
    valueT = nc.dram_tensor("valueT", [H, SKV], bf16, kind="ExternalInput").ap()
    wqT = nc.dram_tensor("wqT", [H, H], bf16, kind="ExternalInput").ap()
    wkT = nc.dram_tensor("wkT", [H, H], bf16, kind="ExternalInput").ap()
    wvT = nc.dram_tensor("wvT", [H, H], bf16, kind="ExternalInput").ap()
    bq2 = nc.dram_tensor("bq2", [EC, P], f32, kind="ExternalInput").ap()
    bk2 = nc.dram_tensor("bk2", [EC, P], f32, kind="ExternalInput").ap()
    bv4 = nc.dram_tensor("bv4", [1, 4 * H], f32, kind="ExternalInput").ap()
    out = nc.dram_tensor("out", [SQ, H], f32, kind="ExternalOutput").ap()

    aps = (queryT, keyT, valueT, wqT, wkT, wvT, bq2, bk2, bv4, out)
    with tile.TileContext(nc) as tc, ExitStack() as ctx:
        _emit(ctx, tc, aps)
    nc.compile()
    return nc


def _get_nc():
    if "nc" not in _CACHE:
        _CACHE["nc"] = _build()
    return _CACHE["nc"]


def _in_maps(query, key, value, Wq, bq, Wk, bk, Wv, bv):
    import ml_dtypes

    bf = ml_dtypes.bfloat16
    q = np.asarray(query, np.float32)
    k = np.asarray(key, np.float32)
    v = np.asarray(value, np.float32)
    # [B, s, d] -> [B, d, s] bf16 layout prep, done host-side as part of
    # sharding (contraction dim on partitions; bf16 halves the DMA bytes).
    qT = np.ascontiguousarray(q.transpose(0, 2, 1)).astype(bf)
    kT = np.ascontiguousarray(k.transpose(0, 2, 1)).astype(bf)
    vT = np.ascontiguousarray(v.transpose(0, 2, 1)).astype(bf)
    wqT = np.ascontiguousarray(np.asarray(Wq, np.float32).T).astype(bf)
    wkT = np.ascontiguousarray(np.asarray(Wk, np.float32).T).astype(bf)
    wvT = np.ascontiguousarray(np.asarray(Wv, np.float32).T).astype(bf)
    bq2 = np.ascontiguousarray(np.asarray(bq, np.float32).reshape(EC, P))
    bk2 = np.ascontiguousarray(np.asarray(bk, np.float32).reshape(EC, P))
    bv4 = np.ascontiguousarray(
        np.tile(np.asarray(bv, np.float32).reshape(1, H), (1, 4))
    )
    maps = []
    for b in range(B):
        maps.append(
            {
                "queryT": qT[b],
                "keyT": kT[b],
                "valueT": vT[b],
                "wqT": wqT,
                "wkT": wkT,
                "wvT": wvT,
                "bq2": bq2,
                "bk2": bk2,
                "bv4": bv4,
            }
        )
    return maps


def _run(in_maps, trace=False, **kw):
    import concourse.bass_utils as bass_utils

    if trace:
        # zero-egress container: skip the artifact upload step
        bass_utils.upload_artifacts = lambda tmpdir: f"local://{tmpdir}"
    nc = _get_nc()
    return bass_utils.run_bass_kernel_spmd(
        nc, in_maps, list(range(N_CORES)), trace=trace, **kw
    )


def kernel(query, key, value, Wq, bq, Wk, bk, Wv, bv):
    in_maps = _in_maps(query, key, value, Wq, bq, Wk, bk, Wv, bv)
    _run(in_maps)  # warmup execution (cold-start insurance)
    res = _run(in_maps)
    return np.stack([res.results[b]["out"] for b in range(B)], axis=0)


# revision 14
# speedup vs baseline: 1.3566x; 1.0927x over previous
"""Cross-attention Trainium2 kernel (v3: bf16 + fp8-DoubleRow scores).

Problem: B=8, SQ=SKV=2048, HIDDEN=256, fp32.
  Q = query @ Wq.T + bq ; K = key @ Wk.T + bk ; V = value @ Wv.T + bv
  out = softmax(Q @ K.T / sqrt(128)) @ V

Sharding: data-parallel over batch - one batch element per NeuronCore,
8 cores, no collectives.

Numerics (validated vs the reference on CPU + HW, harness gate 2e-2):
  inputs/weights bf16, projections bf16, Q/K quantized to fp8e4m3 for
  the scores matmul (fp8 DoubleRow, 256-deep contraction per pass),
  exp partly on ACT (Exp LUT) and partly on DVE (Schraudolph int16
  bit trick: bits of bf16(exp(x)) ~ x*2^7/ln2 + 16248.6+0.5), U and V
  in bf16 for the AV matmul.  Measured HW rel err ~1.2e-2.

Layout/scheduling notes:
  - whole K/V/Q tensors DMA'd up front, one instruction each, spread
    over the SP and Pool DMA queues (per-block loads cost ~1us of
    descriptor generation each and serialized the first 15us).
  - PE emission order per q-block: projection(qb), AV(qb-1),
    scores(qb).  AV covers the DVE latency of the qt8 eviction so the
    PE never stalls at the block boundary (HAM re-throttle).
  - V projection lands 4 k-chunks in one [128,1024] PSUM tile and is
    evicted by a single DVE tensor_tensor (+bv, wide bias tile) per
    group: 4 instructions instead of 16.
  - softmax denominator = ones-columns of Vb (col 256; col 257 pads
    the free dim even); finalize = DVE reciprocal + ACT copy-scale
    (DVE reading its own just-produced reciprocal crashes the device).
"""

import numpy as np

B, SQ, SKV, H = 8, 2048, 2048, 256
SCALE = float(np.sqrt(H / 2.0))
N_CORES = 8

P = 128          # partitions
DC = H // P      # d chunks (2)
EC = H // P      # e chunks (2)
NB = SQ // 512   # 512-row seq blocks (4)
KC = SKV // P    # k chunks (16)
NPAIR = KC // 2  # kc pairs per q block (8)
N_SCHR = 3       # pairs per block on DVE via Schraudolph (of 8)

# Schraudolph constants for bf16-bit output (int16 convert truncates;
# +0.5 recenters; -7.40 zero-means the piecewise-linear error)
SCHR_S = 128.0 / float(np.log(2.0))
SCHR_B = 127.0 * 128.0 - 7.40 + 0.5

_CACHE: dict = {}


def _emit(ctx, tc, aps):
    from concourse import mybir

    nc = tc.nc
    f32 = mybir.dt.float32
    bf16 = mybir.dt.bfloat16
    fp8 = mybir.dt.float8e4
    i16 = mybir.dt.int16
    AF = mybir.ActivationFunctionType
    DR = mybir.MatmulPerfMode.DoubleRow
    queryT, keyT, valueT, wqT, wkT, wvT, bq2, bk2, bv4 = aps[:9]
    out = aps[9]
    inv_scale = 1.0 / SCALE

    const_pool = ctx.enter_context(tc.tile_pool(name="const", bufs=1))
    ktv_pool = ctx.enter_context(tc.tile_pool(name="ktv", bufs=1))
    qt_pool = ctx.enter_context(tc.tile_pool(name="qt", bufs=2))
    u_pool = ctx.enter_context(tc.tile_pool(name="u", bufs=18))
    out_pool = ctx.enter_context(tc.tile_pool(name="outp", bufs=4))
    rec_pool = ctx.enter_context(tc.tile_pool(name="rec", bufs=4))
    # PSUM budget (8 banks of 2KB): pst 2x2 + proj 2x1 + pav 2x1 banks
    ps_s = ctx.enter_context(tc.tile_pool(name="ps_s", bufs=2, space="PSUM"))
    ps_p = ctx.enter_context(tc.tile_pool(name="ps_p", bufs=2, space="PSUM"))
    ps_av = ctx.enter_context(tc.tile_pool(name="ps_av", bufs=2, space="PSUM"))

    # ---- input DMAs: per-512-block, all on the one SP HWDGE queue ----
    # A single queue executes transfers IN ORDER, so each block gets the
    # full HBM bandwidth and the first K block lands ~1.5us in (three
    # whole-tensor DMAs would share bandwidth three ways and delay the
    # first matmul to ~14us).  Order: K blocks, Q block 0 (needed by the
    # early Q projection), V blocks, remaining Q blocks.
    def load_block(src, blk, tag):
        t = const_pool.tile([P, DC, 512], bf16, tag=f"{tag}{blk}")
        nc.sync.dma_start(
            t, src[:, blk * 512:(blk + 1) * 512].rearrange("(c p) s -> p c s", p=P)
        )
        return t

    kin_t = [load_block(keyT, blk, "kin") for blk in range(NB)]
    qin_t = [None] * NB
    qin_t[0] = load_block(queryT, 0, "qin")
    vin_t = [load_block(valueT, blk, "vin") for blk in range(NB)]
    for blk in range(1, NB):
        qin_t[blk] = load_block(queryT, blk, "qin")

    def load_weight(name, src_ap):
        w = const_pool.tile([P, DC, H], bf16, tag=name)
        nc.scalar.dma_start(w, src_ap.rearrange("(c p) e -> p c e", p=P))
        return w

    wk_sb = load_weight("wk", wkT)
    wv_sb = load_weight("wv", wvT)
    wq_sb = load_weight("wq", wqT)

    bq_sb = const_pool.tile([P, EC], f32)
    nc.scalar.dma_start(bq_sb, bq2.rearrange("c p -> p c"))
    bk_sb = const_pool.tile([P, EC], f32)
    nc.scalar.dma_start(bk_sb, bk2.rearrange("c p -> p c"))
    bv_row = const_pool.tile([1, 4 * H], f32)
    nc.scalar.dma_start(bv_row, bv4)
    bv_rep = const_pool.tile([P, 4 * H], f32)
    nc.gpsimd.partition_broadcast(bv_rep, bv_row)

    # ---- persistent per-core tensors ----
    KT8 = ktv_pool.tile([P, EC, SKV], fp8)      # [e_part, ec, k]
    Vb = ktv_pool.tile([P, KC, H + 2], bf16)    # [k_part, kc, e | one one]
    for kc in range(KC):
        nc.vector.tensor_scalar(
            Vb[:, kc, H:H + 2], bv_rep[:, 0:2], 0.0, 1.0,
            mybir.AluOpType.mult, mybir.AluOpType.add,
        )

    # ---- query blocks: pipelined proj -> AV(prev) -> scores -> exp ----
    def emit_proj(qb):
        qt8 = qt_pool.tile([P, EC, 512], fp8, tag="qt8")
        for ec in range(EC):
            pq = ps_p.tile([P, 512], f32, tag="ps_p")
            for dc in range(DC):
                nc.tensor.matmul(
                    pq,
                    lhsT=wq_sb[:, dc, ec * P:(ec + 1) * P],
                    rhs=qin_t[qb][:, dc, :],
                    start=(dc == 0),
                    stop=(dc == DC - 1),
                )
            nc.vector.tensor_scalar(
                qt8[:, ec, :], pq, bq_sb[:, ec:ec + 1], None, mybir.AluOpType.add,
            )
        return qt8

    def emit_scores_exp(qb, qt8):
        us = []
        for g in range(NPAIR):
            pst = ps_s.tile([P, 1024], f32, tag="pst")
            for hh in range(2):
                kc = 2 * g + hh
                nc.tensor.matmul(
                    pst[:, hh * 512:(hh + 1) * 512],
                    lhsT=KT8[:, :, kc * P:(kc + 1) * P],
                    rhs=qt8,
                    start=True,
                    stop=True,
                    perf_mode=DR,
                )
            if g < NPAIR - N_SCHR:
                u = u_pool.tile([P, 1024], bf16, tag="u")
                nc.scalar.activation(u, pst, AF.Exp, scale=inv_scale)
            else:
                u16 = u_pool.tile([P, 1024], i16, tag="u16")
                nc.vector.tensor_scalar(
                    u16, pst, SCHR_S * inv_scale, SCHR_B,
                    mybir.AluOpType.mult, mybir.AluOpType.add,
                )
                u = u16.bitcast(bf16)
            us.append(u)
        return us

    def emit_av(qb, us):
        for qs in range(4):
            pav = ps_av.tile([P, H + 2], f32, tag="pav")
            for g in range(NPAIR):
                u = us[g]
                for hh in range(2):
                    kc = 2 * g + hh
                    nc.tensor.matmul(
                        pav,
                        lhsT=u[:, hh * 512 + qs * P: hh * 512 + (qs + 1) * P],
                        rhs=Vb[:, kc, :],
                        start=(kc == 0),
                        stop=(kc == KC - 1),
                    )
            rec = rec_pool.tile([P, 1], f32, tag="rec")
            nc.vector.reciprocal(rec, pav[:, H:H + 1])
            ot = out_pool.tile([P, H], f32, tag="ot")
            nc.scalar.activation(ot, pav[:, 0:H], AF.Copy, scale=rec)
            nc.sync.dma_start(
                out[qb * 512 + qs * P: qb * 512 + (qs + 1) * P, :], ot
            )

    # ---- key: project into KT8 (fp8, bias fused) ----
    for blk in range(NB):
        cols = slice(blk * 512, (blk + 1) * 512)
        for ec in range(EC):
            pk = ps_p.tile([P, 512], f32, tag="ps_p")
            for dc in range(DC):
                nc.tensor.matmul(
                    pk,
                    lhsT=wk_sb[:, dc, ec * P:(ec + 1) * P],
                    rhs=kin_t[blk][:, dc, :],
                    start=(dc == 0),
                    stop=(dc == DC - 1),
                )
            nc.vector.tensor_scalar(
                KT8[:, ec, cols],
                pk, bk_sb[:, ec:ec + 1], None, mybir.AluOpType.add,
            )

    # ---- Q projection of block 0, early: its DVE eviction drains
    # while the PE runs the V projection, so scores(0) start stall-free
    qt8_next = emit_proj(0)

    # ---- value: project into Vb (+bv), 4 k-chunks per PSUM tile ----
    for blk in range(NB):
        pv4 = ps_s.tile([P, 1024], f32, tag="pst")
        for j in range(4):
            kc = blk * 4 + j
            for dc in range(DC):
                nc.tensor.matmul(
                    pv4[:, j * H:(j + 1) * H],
                    lhsT=vin_t[blk][:, dc, j * P:(j + 1) * P],
                    rhs=wv_sb[:, dc, :],
                    start=(dc == 0),
                    stop=(dc == DC - 1),
                )
        nc.vector.tensor_add(
            Vb[:, blk * 4:(blk + 1) * 4, 0:H], pv4, bv_rep,
        )

    # Pipeline: qt8(qb) is always evicted one iteration ahead, so
    # scores(qb) start with no DVE wait; AV(qb-1) fills the PE while
    # ACT/DVE chew through exp(qb).
    pending = None  # (qb, us) awaiting AV+finalize
    for qb in range(NB):
        qt8 = qt8_next
        us = emit_scores_exp(qb, qt8)
        if qb + 1 < NB:
            qt8_next = emit_proj(qb + 1)
        if pending is not None:
            emit_av(*pending)
        pending = (qb, us)
    emit_av(*pending)


def _build():
    from contextlib import ExitStack

    import concourse.tile as tile
    from concourse import bacc, mybir

    f32 = mybir.dt.float32
    bf16 = mybir.dt.bfloat16
    nc = bacc.Bacc(
        "TRN2", target_bir_lowering=False, debug=False, num_devices=N_CORES
    )
    queryT = nc.dram_tensor("queryT", [H, SQ], bf16, kind="ExternalInput").ap()
    keyT = nc.dram_tensor("keyT", [H, SKV], bf16, kind="ExternalInput").ap()
    valueT = nc.dram_tensor("valueT", [H, SKV], bf16, kind="ExternalInput").ap()
    wqT = nc.dram_tensor("wqT", [H, H], bf16, kind="ExternalInput").ap()
    wkT = nc.dram_tensor("wkT", [H, H], bf16, kind="ExternalInput").ap()
    wvT = nc.dram_tensor("wvT", [H, H], bf16, kind="ExternalInput").ap()
    bq2 = nc.dram_tensor("bq2", [EC, P], f32, kind="ExternalInput").ap()
    bk2 = nc.dram_tensor("bk2", [EC, P], f32, kind="ExternalInput").ap()
    bv4 = nc.dram_tensor("bv4", [1, 4 * H], f32, kind="ExternalInput").ap()
    out = nc.dram_tensor("out", [SQ, H], f32, kind="ExternalOutput").ap()

    aps = (queryT, keyT, valueT, wqT, wkT, wvT, bq2, bk2, bv4, out)
    with tile.TileContext(nc) as tc, ExitStack() as ctx:
        _emit(ctx, tc, aps)
    nc.compile()
    return nc


def _get_nc():
    if "nc" not in _CACHE:
        _CACHE["nc"] = _build()
    return _CACHE["nc"]


def _in_maps(query, key, value, Wq, bq, Wk, bk, Wv, bv):
    import ml_dtypes

    bf = ml_dtypes.bfloat16
    q = np.asarray(query, np.float32)
    k = np.asarray(key, np.float32)
    v = np.asarray(value, np.float32)
    # [B, s, d] -> [B, d, s] bf16 layout prep, done host-side as part of
    # sharding (contraction dim on partitions; bf16 halves the DMA bytes).
    qT = np.ascontiguousarray(q.transpose(0, 2, 1)).astype(bf)
    kT = np.ascontiguousarray(k.transpose(0, 2, 1)).astype(bf)
    vT = np.ascontiguousarray(v.transpose(0, 2, 1)).astype(bf)
    wqT = np.ascontiguousarray(np.asarray(Wq, np.float32).T).astype(bf)
    wkT = np.ascontiguousarray(np.asarray(Wk, np.float32).T).astype(bf)
    wvT = np.ascontiguousarray(np.asarray(Wv, np.float32).T).astype(bf)
    bq2 = np.ascontiguousarray(np.asarray(bq, np.float32).reshape(EC, P))
    bk2 = np.ascontiguousarray(np.asarray(bk, np.float32).reshape(EC, P))
    bv4 = np.ascontiguousarray(
        np.tile(np.asarray(bv, np.float32).reshape(1, H), (1, 4))
    )
    maps = []
    for b in range(B):
        maps.append(
            {
                "queryT": qT[b],
                "keyT": kT[b],
                "valueT": vT[b],
                "wqT": wqT,
                "wkT": wkT,
                "wvT": wvT,
                "bq2": bq2,
                "bk2": bk2,
                "bv4": bv4,
            }
        )
    return maps


def _run(in_maps, trace=False, **kw):
    import concourse.bass_utils as bass_utils

    if trace:
        # zero-egress container: skip the artifact upload step
        bass_utils.upload_artifacts = lambda tmpdir: f"local://{tmpdir}"
    nc = _get_nc()
    return bass_utils.run_bass_kernel_spmd(
        nc, in_maps, list(range(N_CORES)), trace=trace, **kw
    )


def kernel(query, key, value, Wq, bq, Wk, bk, Wv, bv):
    in_maps = _in_maps(query, key, value, Wq, bq, Wk, bk, Wv, bv)
    _run(in_maps)  # warmup execution (cold-start insurance)
    res = _run(in_maps)
    return np.stack([res.results[b]["out"] for b in range(B)], axis=0)


# revision 15
# speedup vs baseline: 1.4331x; 1.0564x over previous
"""Cross-attention Trainium2 kernel (v3: bf16 + fp8-DoubleRow scores).

Problem: B=8, SQ=SKV=2048, HIDDEN=256, fp32.
  Q = query @ Wq.T + bq ; K = key @ Wk.T + bk ; V = value @ Wv.T + bv
  out = softmax(Q @ K.T / sqrt(128)) @ V

Sharding: data-parallel over batch - one batch element per NeuronCore,
8 cores, no collectives.

Numerics (validated vs the reference on CPU + HW, harness gate 2e-2):
  inputs/weights bf16, projections bf16, Q/K quantized to fp8e4m3 for
  the scores matmul (fp8 DoubleRow, 256-deep contraction per pass),
  exp partly on ACT (Exp LUT) and partly on DVE (Schraudolph int16
  bit trick: bits of bf16(exp(x)) ~ x*2^7/ln2 + 16248.6+0.5), U and V
  in bf16 for the AV matmul.  Measured HW rel err ~1.2e-2.

Layout/scheduling notes:
  - whole K/V/Q tensors DMA'd up front, one instruction each, spread
    over the SP and Pool DMA queues (per-block loads cost ~1us of
    descriptor generation each and serialized the first 15us).
  - PE emission order per q-block: projection(qb), AV(qb-1),
    scores(qb).  AV covers the DVE latency of the qt8 eviction so the
    PE never stalls at the block boundary (HAM re-throttle).
  - V projection lands 4 k-chunks in one [128,1024] PSUM tile and is
    evicted by a single DVE tensor_tensor (+bv, wide bias tile) per
    group: 4 instructions instead of 16.
  - softmax denominator = ones-columns of Vb (col 256; col 257 pads
    the free dim even); finalize = DVE reciprocal + ACT copy-scale
    (DVE reading its own just-produced reciprocal crashes the device).
"""

import numpy as np

B, SQ, SKV, H = 8, 2048, 2048, 256
SCALE = float(np.sqrt(H / 2.0))
N_CORES = 8

P = 128          # partitions
DC = H // P      # d chunks (2)
EC = H // P      # e chunks (2)
NB = SQ // 512   # 512-row seq blocks (4)
KC = SKV // P    # k chunks (16)
NPAIR = KC // 2  # kc pairs per q block (8)
N_SCHR = 3       # pairs per block on DVE via Schraudolph (of 8)

# Schraudolph constants for bf16-bit output (int16 convert truncates;
# +0.5 recenters; -7.40 zero-means the piecewise-linear error)
SCHR_S = 128.0 / float(np.log(2.0))
SCHR_B = 127.0 * 128.0 - 7.40 + 0.5

_CACHE: dict = {}


def _emit(ctx, tc, aps):
    from concourse import mybir

    nc = tc.nc
    f32 = mybir.dt.float32
    bf16 = mybir.dt.bfloat16
    fp8 = mybir.dt.float8e4
    i16 = mybir.dt.int16
    AF = mybir.ActivationFunctionType
    DR = mybir.MatmulPerfMode.DoubleRow
    queryT, keyT, valueT, wqT, wkT, wvT, bq2, bk2, bv4 = aps[:9]
    out = aps[9]
    inv_scale = 1.0 / SCALE

    const_pool = ctx.enter_context(tc.tile_pool(name="const", bufs=1))
    ktv_pool = ctx.enter_context(tc.tile_pool(name="ktv", bufs=1))
    qt_pool = ctx.enter_context(tc.tile_pool(name="qt", bufs=2))
    u_pool = ctx.enter_context(tc.tile_pool(name="u", bufs=18))
    out_pool = ctx.enter_context(tc.tile_pool(name="outp", bufs=4))
    rec_pool = ctx.enter_context(tc.tile_pool(name="rec", bufs=4))
    # PSUM budget (8 banks of 2KB): pst 2x2 + proj 2x1 + pav 2x1 banks
    ps_s = ctx.enter_context(tc.tile_pool(name="ps_s", bufs=2, space="PSUM"))
    ps_p = ctx.enter_context(tc.tile_pool(name="ps_p", bufs=2, space="PSUM"))
    ps_av = ctx.enter_context(tc.tile_pool(name="ps_av", bufs=2, space="PSUM"))

    # ---- input DMAs: per-512-block, all on the one SP HWDGE queue ----
    # A single queue executes transfers IN ORDER, so each block gets the
    # full HBM bandwidth and the first K block lands ~1.5us in (three
    # whole-tensor DMAs would share bandwidth three ways and delay the
    # first matmul to ~14us).  Order: K blocks, Q block 0 (needed by the
    # early Q projection), V blocks, remaining Q blocks.
    def load_block(src, blk, tag):
        t = const_pool.tile([P, DC, 512], bf16, tag=f"{tag}{blk}")
        nc.sync.dma_start(
            t, src[:, blk * 512:(blk + 1) * 512].rearrange("(c p) s -> p c s", p=P)
        )
        return t

    def load_weight(name, src_ap):
        w = const_pool.tile([P, DC, H], bf16, tag=name)
        nc.sync.dma_start(w, src_ap.rearrange("(c p) e -> p c e", p=P))
        return w

    wk_sb = load_weight("wk", wkT)
    bk_sb = const_pool.tile([P, EC], f32)
    nc.sync.dma_start(bk_sb, bk2.rearrange("c p -> p c"))
    kin_t = [load_block(keyT, blk, "kin") for blk in range(NB)]
    wq_sb = load_weight("wq", wqT)
    bq_sb = const_pool.tile([P, EC], f32)
    nc.sync.dma_start(bq_sb, bq2.rearrange("c p -> p c"))
    qin_t = [None] * NB
    qin_t[0] = load_block(queryT, 0, "qin")
    wv_sb = load_weight("wv", wvT)
    bv_row = const_pool.tile([1, 4 * H], f32)
    nc.sync.dma_start(bv_row, bv4)
    vin_t = [load_block(valueT, blk, "vin") for blk in range(NB)]
    for blk in range(1, NB):
        qin_t[blk] = load_block(queryT, blk, "qin")
    bv_rep = const_pool.tile([P, 4 * H], f32)
    nc.gpsimd.partition_broadcast(bv_rep, bv_row)

    # ---- persistent per-core tensors ----
    KT8 = ktv_pool.tile([P, EC, SKV], fp8)      # [e_part, ec, k]
    Vb = ktv_pool.tile([P, KC, H + 2], bf16)    # [k_part, kc, e | one one]
    for kc in range(KC):
        nc.vector.tensor_scalar(
            Vb[:, kc, H:H + 2], bv_rep[:, 0:2], 0.0, 1.0,
            mybir.AluOpType.mult, mybir.AluOpType.add,
        )

    # ---- query blocks: pipelined proj -> AV(prev) -> scores -> exp ----
    def emit_proj(qb):
        qt8 = qt_pool.tile([P, EC, 512], fp8, tag="qt8")
        for ec in range(EC):
            pq = ps_p.tile([P, 512], f32, tag="ps_p")
            for dc in range(DC):
                nc.tensor.matmul(
                    pq,
                    lhsT=wq_sb[:, dc, ec * P:(ec + 1) * P],
                    rhs=qin_t[qb][:, dc, :],
                    start=(dc == 0),
                    stop=(dc == DC - 1),
                )
            nc.vector.tensor_scalar(
                qt8[:, ec, :], pq, bq_sb[:, ec:ec + 1], None, mybir.AluOpType.add,
            )
        return qt8

    def emit_scores_pair(qb, qt8, g, us):
        pst = ps_s.tile([P, 1024], f32, tag="pst")
        for hh in range(2):
            kc = 2 * g + hh
            nc.tensor.matmul(
                pst[:, hh * 512:(hh + 1) * 512],
                lhsT=KT8[:, :, kc * P:(kc + 1) * P],
                rhs=qt8,
                start=True,
                stop=True,
                perf_mode=DR,
            )
        if g < NPAIR - N_SCHR:
            u = u_pool.tile([P, 1024], bf16, tag="u")
            nc.scalar.activation(u, pst, AF.Exp, scale=inv_scale)
        else:
            u16 = u_pool.tile([P, 1024], i16, tag="u16")
            nc.vector.tensor_scalar(
                u16, pst, SCHR_S * inv_scale, SCHR_B,
                mybir.AluOpType.mult, mybir.AluOpType.add,
            )
            u = u16.bitcast(bf16)
        us.append(u)

    def emit_av_chunk(qb, us, qs):
        pav = ps_av.tile([P, H + 2], f32, tag="pav")
        for g in range(NPAIR):
            u = us[g]
            for hh in range(2):
                kc = 2 * g + hh
                nc.tensor.matmul(
                    pav,
                    lhsT=u[:, hh * 512 + qs * P: hh * 512 + (qs + 1) * P],
                    rhs=Vb[:, kc, :],
                    start=(kc == 0),
                    stop=(kc == KC - 1),
                )
        rec = rec_pool.tile([P, 1], f32, tag="rec")
        nc.vector.reciprocal(rec, pav[:, H:H + 1])
        ot = out_pool.tile([P, H], f32, tag="ot")
        nc.scalar.activation(ot, pav[:, 0:H], AF.Copy, scale=rec)
        nc.sync.dma_start(
            out[qb * 512 + qs * P: qb * 512 + (qs + 1) * P, :], ot
        )

    # ---- key: project into KT8 (fp8, bias fused) ----
    for blk in range(NB):
        cols = slice(blk * 512, (blk + 1) * 512)
        for ec in range(EC):
            pk = ps_p.tile([P, 512], f32, tag="ps_p")
            for dc in range(DC):
                nc.tensor.matmul(
                    pk,
                    lhsT=wk_sb[:, dc, ec * P:(ec + 1) * P],
                    rhs=kin_t[blk][:, dc, :],
                    start=(dc == 0),
                    stop=(dc == DC - 1),
                )
            nc.vector.tensor_scalar(
                KT8[:, ec, cols],
                pk, bk_sb[:, ec:ec + 1], None, mybir.AluOpType.add,
            )

    # ---- Q projection of block 0, early: its DVE eviction drains
    # while the PE runs the V projection, so scores(0) start stall-free
    qt8_next = emit_proj(0)

    # ---- value: project into Vb (+bv), 4 k-chunks per PSUM tile ----
    for blk in range(NB):
        pv4 = ps_s.tile([P, 1024], f32, tag="pst")
        for j in range(4):
            kc = blk * 4 + j
            for dc in range(DC):
                nc.tensor.matmul(
                    pv4[:, j * H:(j + 1) * H],
                    lhsT=vin_t[blk][:, dc, j * P:(j + 1) * P],
                    rhs=wv_sb[:, dc, :],
                    start=(dc == 0),
                    stop=(dc == DC - 1),
                )
        nc.vector.tensor_add(
            Vb[:, blk * 4:(blk + 1) * 4, 0:H], pv4, bv_rep,
        )

    # Pipeline: qt8(qb) is evicted one iteration ahead so scores(qb)
    # start with no DVE wait.  AV chunks of qb-1 are INTERLEAVED between
    # scores pairs: the in-order PE queue then always has ready matmuls
    # while the pst ring (2 bufs) throttles scores to ACT/DVE exp pace.
    pending = None  # (qb, us) awaiting AV+finalize
    for qb in range(NB):
        qt8 = qt8_next
        us = []
        for g in range(NPAIR):
            emit_scores_pair(qb, qt8, g, us)
            if g % 2 == 1 and pending is not None:
                emit_av_chunk(pending[0], pending[1], g // 2)
        if qb + 1 < NB:
            qt8_next = emit_proj(qb + 1)
        pending = (qb, us)
    for qs in range(4):
        emit_av_chunk(pending[0], pending[1], qs)


def _build():
    from contextlib import ExitStack

    import concourse.tile as tile
    from concourse import bacc, mybir

    f32 = mybir.dt.float32
    bf16 = mybir.dt.bfloat16
    nc = bacc.Bacc(
        "TRN2", target_bir_lowering=False, debug=False, num_devices=N_CORES
    )
    queryT = nc.dram_tensor("queryT", [H, SQ], bf16, kind="ExternalInput").ap()
    keyT = nc.dram_tensor("keyT", [H, SKV], bf16, kind="ExternalInput").ap()
    valueT = nc.dram_tensor("valueT", [H, SKV], bf16, kind="ExternalInput").ap()
    wqT = nc.dram_tensor("wqT", [H, H], bf16, kind="ExternalInput").ap()
    wkT = nc.dram_tensor("wkT", [H, H], bf16, kind="ExternalInput").ap()
    wvT = nc.dram_tensor("wvT", [H, H], bf16, kind="ExternalInput").ap()
    bq2 = nc.dram_tensor("bq2", [EC, P], f32, kind="ExternalInput").ap()
    bk2 = nc.dram_tensor("bk2", [EC, P], f32, kind="ExternalInput").ap()
    bv4 = nc.dram_tensor("bv4", [1, 4 * H], f32, kind="ExternalInput").ap()
    out = nc.dram_tensor("out", [SQ, H], f32, kind="ExternalOutput").ap()

    aps = (queryT, keyT, valueT, wqT, wkT, wvT, bq2, bk2, bv4, out)
    with tile.TileContext(nc) as tc, ExitStack() as ctx:
        _emit(ctx, tc, aps)
    nc.compile()
    return nc


def _get_nc():
    if "nc" not in _CACHE:
        _CACHE["nc"] = _build()
    return _CACHE["nc"]


def _in_maps(query, key, value, Wq, bq, Wk, bk, Wv, bv):
    import ml_dtypes

    bf = ml_dtypes.bfloat16
    q = np.asarray(query, np.float32)
    k = np.asarray(key, np.float32)
    v = np.asarray(value, np.float32)
    # [B, s, d] -> [B, d, s] bf16 layout prep, done host-side as part of
    # sharding (contraction dim on partitions; bf16 halves the DMA bytes).
    qT = np.ascontiguousarray(q.transpose(0, 2, 1)).astype(bf)
    kT = np.ascontiguousarray(k.transpose(0, 2, 1)).astype(bf)
    vT = np.ascontiguousarray(v.transpose(0, 2, 1)).astype(bf)
    wqT = np.ascontiguousarray(np.asarray(Wq, np.float32).T).astype(bf)
    wkT = np.ascontiguousarray(np.asarray(Wk, np.float32).T).astype(bf)
    wvT = np.ascontiguousarray(np.asarray(Wv, np.float32).T).astype(bf)
    bq2 = np.ascontiguousarray(np.asarray(bq, np.float32).reshape(EC, P))
    bk2 = np.ascontiguousarray(np.asarray(bk, np.float32).reshape(EC, P))
    bv4 = np.ascontiguousarray(
        np.tile(np.asarray(bv, np.float32).reshape(1, H), (1, 4))
    )
    maps = []
    for b in range(B):
        maps.append(
            {
                "queryT": qT[b],
                "keyT": kT[b],
                "valueT": vT[b],
                "wqT": wqT,
                "wkT": wkT,
                "wvT": wvT,
                "bq2": bq2,
                "bk2": bk2,
                "bv4": bv4,
            }
        )
    return maps


def _run(in_maps, trace=False, **kw):
    import concourse.bass_utils as bass_utils

    if trace:
        # zero-egress container: skip the artifact upload step
        bass_utils.upload_artifacts = lambda tmpdir: f"local://{tmpdir}"
    nc = _get_nc()
    return bass_utils.run_bass_kernel_spmd(
        nc, in_maps, list(range(N_CORES)), trace=trace, **kw
    )


def kernel(query, key, value, Wq, bq, Wk, bk, Wv, bv):
    in_maps = _in_maps(query, key, value, Wq, bq, Wk, bk, Wv, bv)
    _run(in_maps)  # warmup execution (cold-start insurance)
    res = _run(in_maps)
    return np.stack([res.results[b]["out"] for b in range(B)], axis=0)
